# revision 27
# baseline (speedup 1.0000x reference)
"""Bahdanau additive attention on 8 Trainium2 NeuronCores.

Problem shapes (hardcoded):
  encoder_out_seq [B=4, Te=512, He=128] f32
  decoder_out_seq [B=4, Td=256, Hd=256] f32
  W_a [128, 128], U_a [256, 128], V_a [128, 1] f32
Returns (c [B, Td, He], e [B, Td, Te]) matching the jax reference.

Sharding: 8 cores = batch (4) x decoder-half (2). Each core computes a
[Td_core=128] slice of decoder steps for one batch element; weights and the
encoder sequence are replicated per core. No collectives. The host-side shard
step also hands each core transposed copies of its encoder/decoder slices
(encT, decT) — a pure layout choice that removes on-device PE transposes from
the startup critical path.

Per-core algorithm (Tile-scheduled):
  WsT[f,s]  = W_a^T @ encT             (PE, fp32; result cast to bf16)
  UhT[f,t]  = U_a^T @ decT             (PE, fp32)
  Z[f,(t,s)] = WsT[f,s] + UhT[f,t]     (DVE tensor_scalar, bf16 out)
  feat      = tanh(Z) -> bf16          (ACT, big tiles; the 8.4M-elem floor)
  eT[s,t]   = feat_t^T @ v             (PE, N=1 column matmuls into PSUM)
  softmax over s without max-subtraction (scores are O(1) by construction).
  The decoder range is processed in four quarters, each with its own PSUM
  score region and its own epilogue, so earlier quarters' softmax/c-matmul/DMA
  overlap later quarters' tanh pipeline and only the last quarter's epilogue
  is exposed as tail:
    exp on ACT, PE transposes to [t,s], DVE row-reduce + reciprocal,
    c = (exp_eT)^T @ enc * (1/denom), e = exp * (1/denom).
"""

import ml_dtypes
import numpy as np

import concourse.bass as bass
import concourse.bacc as bacc
import concourse.tile as tile
from concourse import mybir
from concourse.bass_utils import run_bass_kernel_spmd
from concourse.masks import make_identity

B, TE, TD, HE, HD = 4, 512, 256, 128, 256
N_CORES = 8
TD_CORE = TD // 2  # 128 decoder steps per core
S_TILES = TE // 128  # 4

# t-block sizes per quarter: small blocks at the very start (get ACT busy
# early) and the very end (short tail before the final epilogue).
# (t0, size, blocks) per chunk; the last chunk is small so the final
# epilogue chain after the last tanh is short.
QUARTERS = [
    (0, 32, [4, 4, 8, 16]),
    (32, 32, [16, 16]),
    (64, 32, [16, 16]),
    (96, 32, [16, 8, 4, 4]),
]
assert QUARTERS[-1][0] + QUARTERS[-1][1] == TD_CORE

F32 = mybir.dt.float32
BF16 = mybir.dt.bfloat16
AF = mybir.ActivationFunctionType


def _build():
    from contextlib import ExitStack

    nc = bacc.Bacc("TRN2", target_bir_lowering=False, debug=False)

    # Host-packed inputs (one DMA each; see make_in_maps):
    #   packA [128, 644] = encT (512) | W_a (128) | V_a (1) | pad (3)
    #   packB [128, 512] = decT partition-packed (256) | U_a partition-packed (256)
    #   encP  [128, 512] = enc partition-packed: encP[p, 128*i+e] = enc[128*i+p, e]
    packA_d = nc.dram_tensor("packA", [128, TE + HE + 4], BF16, kind="ExternalInput").ap()
    packB_d = nc.dram_tensor("packB", [128, 4 * 128], BF16, kind="ExternalInput").ap()
    encP_d = nc.dram_tensor("encP", [128, S_TILES * HE], F32, kind="ExternalInput").ap()
    c_d = nc.dram_tensor("c", [TD_CORE, HE], F32, kind="ExternalOutput").ap()
    e_d = nc.dram_tensor("e", [TD_CORE, TE], F32, kind="ExternalOutput").ap()

    with tile.TileContext(nc) as tc:
        with ExitStack() as ctx:
            consts = ctx.enter_context(tc.tile_pool(name="consts", bufs=1))
            zpool = ctx.enter_context(tc.tile_pool(name="zpool", bufs=6))
            fpool = ctx.enter_context(tc.tile_pool(name="fpool", bufs=2))
            opool = ctx.enter_context(tc.tile_pool(name="opool", bufs=2))
            ps_e = ctx.enter_context(tc.tile_pool(name="ps_e", bufs=1, space="PSUM"))
            ps_out = ctx.enter_context(
                tc.tile_pool(name="ps_out", bufs=2, space="PSUM")
            )

            # ---- load inputs: 3 packed DMAs ----
            packA = consts.tile([128, TE + HE + 4], BF16, tag="packA")
            nc.sync.dma_start(out=packA[:], in_=packA_d[:])
            packB = consts.tile([128, 4 * 128], BF16, tag="packB")
            nc.scalar.dma_start(out=packB[:], in_=packB_d[:])
            encP = consts.tile([128, S_TILES * HE], F32, tag="encP")
            nc.sync.dma_start(out=encP[:], in_=encP_d[:])

            encT_sb = packA[:, 0:TE]  # [e, s]
            wa_sb = packA[:, TE : TE + HE]
            va_sb = packA[:, TE + HE : TE + HE + 1]
            decT_sb = [packB[:, 0:128], packB[:, 128:256]]  # [d, t] halves
            ua_sb = [packB[:, 256:384], packB[:, 384:512]]  # [d, f] halves
            enc_sb = [encP[:, 128 * i : 128 * (i + 1)] for i in range(S_TILES)]

            ident = consts.tile([128, 128], F32, tag="ident")
            make_identity(nc, ident[:])

            v_bf = va_sb  # already bf16 in packA

            wst_sb = consts.tile([HE, TE], BF16, tag="wst_sb")  # [f, s]
            uht_sb = consts.tile([HE, TD_CORE], F32, tag="uht_sb")  # [f, t]
            with tc.tile_pool(name="ps_pre", bufs=1, space="PSUM") as ps_pre:
                wst_ps = ps_pre.tile([HE, TE], F32, tag="wst")
                nc.tensor.matmul(
                    out=wst_ps[:],
                    lhsT=wa_sb,
                    rhs=encT_sb,
                    start=True,
                    stop=True,
                )
                nc.scalar.copy(out=wst_sb[:], in_=wst_ps[:])

                uht_ps = ps_pre.tile([HE, TD_CORE], F32, tag="uht")
                for i in range(HD // 128):
                    nc.tensor.matmul(
                        out=uht_ps[:],
                        lhsT=ua_sb[i],
                        rhs=decT_sb[i],
                        start=(i == 0),
                        stop=(i == HD // 128 - 1),
                    )
                nc.scalar.copy(out=uht_sb[:], in_=uht_ps[:])

            # ---- per-quarter main loop + epilogue ----
            for q, (tq0, TQ, blocks) in enumerate(QUARTERS):
                # e_ps free layout: index i*TQ + tl holds eT[s in tile i, tq0+tl]
                e_ps = ps_e.tile([128, S_TILES * TQ], F32, tag=f"e_ps{q % 2}")
                tl = 0
                for tb in blocks:
                    z = zpool.tile([128, tb * TE], BF16, tag="z")
                    for j in range(tb):
                        nc.vector.tensor_scalar_add(
                            out=z[:, TE * j : TE * (j + 1)],
                            in0=wst_sb[:],
                            scalar1=uht_sb[:, tq0 + tl + j : tq0 + tl + j + 1],
                        )
                    feat = fpool.tile([128, tb * TE], BF16, tag="feat")
                    nc.scalar.activation(out=feat[:], in_=z[:], func=AF.Tanh)
                    for j in range(tb):
                        for i in range(S_TILES):
                            col = TQ * i + tl + j
                            nc.tensor.matmul(
                                out=e_ps[:, col : col + 1],
                                lhsT=feat[
                                    :, TE * j + 128 * i : TE * j + 128 * (i + 1)
                                ],
                                rhs=v_bf,
                                start=True,
                                stop=True,
                            )
                    tl += tb

                # epilogue for this quarter
                expT = opool.tile([128, S_TILES * TQ], F32, tag="expT")
                nc.scalar.activation(out=expT[:], in_=e_ps[:], func=AF.Exp)

                exp_ts = ps_out.tile([TQ, TE], F32, tag="exp_ts")
                for i in range(S_TILES):
                    nc.tensor.transpose(
                        out=exp_ts[:, 128 * i : 128 * (i + 1)],
                        in_=expT[:, TQ * i : TQ * (i + 1)],
                        identity=ident[:],
                    )

                denom = opool.tile([TQ, 1], F32, tag="denom")
                nc.vector.reduce_sum(
                    out=denom[:], in_=exp_ts[:], axis=mybir.AxisListType.X
                )
                rcol = opool.tile([TQ, 1], F32, tag="rcol")
                nc.vector.reciprocal(out=rcol[:], in_=denom[:])

                e_sm = opool.tile([TQ, TE], F32, tag="e_sm")
                nc.vector.tensor_scalar_mul(
                    out=e_sm[:], in0=exp_ts[:], scalar1=rcol[:]
                )
                nc.sync.dma_start(out=e_d[tq0 : tq0 + TQ, :], in_=e_sm[:])

                c_ps = ps_out.tile([TQ, HE], F32, tag="c_ps")
                for i in range(S_TILES):
                    nc.tensor.matmul(
                        out=c_ps[:],
                        lhsT=expT[:, TQ * i : TQ * (i + 1)],
                        rhs=enc_sb[i],
                        start=(i == 0),
                        stop=(i == S_TILES - 1),
                    )
                c_sb = opool.tile([TQ, HE], F32, tag="c_sb")
                nc.vector.tensor_scalar_mul(
                    out=c_sb[:], in0=c_ps[:], scalar1=rcol[:]
                )
                nc.sync.dma_start(out=c_d[tq0 : tq0 + TQ, :], in_=c_sb[:])

    nc.compile()
    return nc


_NC = None


def _get_nc():
    global _NC
    if _NC is None:
        _NC = _build()
    return _NC


def make_in_maps(encoder_out_seq, decoder_out_seq, W_a, U_a, V_a):
    enc = np.ascontiguousarray(np.asarray(encoder_out_seq, dtype=np.float32))
    dec = np.ascontiguousarray(np.asarray(decoder_out_seq, dtype=np.float32))
    wa = np.ascontiguousarray(np.asarray(W_a, dtype=np.float32))
    ua = np.ascontiguousarray(np.asarray(U_a, dtype=np.float32))
    va = np.ascontiguousarray(np.asarray(V_a, dtype=np.float32))
    in_maps = []
    for core in range(N_CORES):
        b, h = core // 2, core % 2
        dec_slice = dec[b, h * TD_CORE : (h + 1) * TD_CORE, :]
        decT = dec_slice.T  # [256 d, 128 t]
        packA = np.concatenate(
            [enc[b].T, wa, va, np.zeros((HE, 3), np.float32)], axis=1
        ).astype(ml_dtypes.bfloat16)  # [128, 644]
        packB = np.concatenate(
            [
                decT.reshape(2, 128, TD_CORE).transpose(1, 0, 2).reshape(128, 256),
                ua.reshape(2, 128, HE).transpose(1, 0, 2).reshape(128, 256),
            ],
            axis=1,
        ).astype(ml_dtypes.bfloat16)  # [128, 512]
        encP = (
            enc[b].reshape(S_TILES, 128, HE).transpose(1, 0, 2).reshape(128, -1)
        )  # [128, 512]
        in_maps.append(
            {
                "packA": np.ascontiguousarray(packA),
                "packB": np.ascontiguousarray(packB),
                "encP": np.ascontiguousarray(encP),
            }
        )
    return in_maps


def assemble(results):
    c = np.zeros((B, TD, HE), dtype=np.float32)
    e = np.zeros((B, TD, TE), dtype=np.float32)
    for core in range(N_CORES):
        b, h = core // 2, core % 2
        c[b, h * TD_CORE : (h + 1) * TD_CORE, :] = results[core]["c"]
        e[b, h * TD_CORE : (h + 1) * TD_CORE, :] = results[core]["e"]
    return c, e


def kernel(encoder_out_seq, decoder_out_seq, W_a, U_a, V_a):
    nc = _get_nc()
    in_maps = make_in_maps(encoder_out_seq, decoder_out_seq, W_a, U_a, V_a)
    res = run_bass_kernel_spmd(nc, in_maps, list(range(N_CORES)))
    return assemble(res.results)


# revision 29
# speedup vs baseline: 1.0024x; 1.0024x over previous
"""Bahdanau additive attention on 8 Trainium2 NeuronCores.

Problem shapes (hardcoded):
  encoder_out_seq [B=4, Te=512, He=128] f32
  decoder_out_seq [B=4, Td=256, Hd=256] f32
  W_a [128, 128], U_a [256, 128], V_a [128, 1] f32
Returns (c [B, Td, He], e [B, Td, Te]) matching the jax reference.

Sharding: 8 cores = batch (4) x decoder-half (2). Each core computes a
[Td_core=128] slice of decoder steps for one batch element; weights and the
encoder sequence are replicated per core. No collectives. The host-side shard
step also hands each core transposed copies of its encoder/decoder slices
(encT, decT) — a pure layout choice that removes on-device PE transposes from
the startup critical path.

Per-core algorithm (Tile-scheduled):
  WsT[f,s]  = W_a^T @ encT             (PE, fp32; result cast to bf16)
  UhT[f,t]  = U_a^T @ decT             (PE, fp32)
  Z[f,(t,s)] = WsT[f,s] + UhT[f,t]     (DVE tensor_scalar, bf16 out)
  feat      = tanh(Z) -> bf16          (ACT, big tiles; the 8.4M-elem floor)
  eT[s,t]   = feat_t^T @ v             (PE, N=1 column matmuls into PSUM)
  softmax over s without max-subtraction (scores are O(1) by construction).
  The decoder range is processed in four quarters, each with its own PSUM
  score region and its own epilogue, so earlier quarters' softmax/c-matmul/DMA
  overlap later quarters' tanh pipeline and only the last quarter's epilogue
  is exposed as tail:
    exp on ACT, PE transposes to [t,s], DVE row-reduce + reciprocal,
    c = (exp_eT)^T @ enc * (1/denom), e = exp * (1/denom).
"""

import ml_dtypes
import numpy as np

import concourse.bass as bass
import concourse.bacc as bacc
import concourse.tile as tile
from concourse import mybir
from concourse.bass_utils import run_bass_kernel_spmd
from concourse.masks import make_identity

B, TE, TD, HE, HD = 4, 512, 256, 128, 256
N_CORES = 8
TD_CORE = TD // 2  # 128 decoder steps per core
S_TILES = TE // 128  # 4

# t-block sizes per quarter: small blocks at the very start (get ACT busy
# early) and the very end (short tail before the final epilogue).
# (t0, size, blocks) per chunk; the last chunk is small so the final
# epilogue chain after the last tanh is short.
QUARTERS = [
    (0, 32, [4, 4, 8, 16]),
    (32, 32, [16, 16]),
    (64, 32, [16, 16]),
    (96, 32, [16, 8, 4, 4]),
]
assert QUARTERS[-1][0] + QUARTERS[-1][1] == TD_CORE

F32 = mybir.dt.float32
BF16 = mybir.dt.bfloat16
AF = mybir.ActivationFunctionType


def _build():
    from contextlib import ExitStack

    nc = bacc.Bacc("TRN2", target_bir_lowering=False, debug=False)

    # Host-packed inputs (one DMA each; see make_in_maps):
    #   packA [128, 644] = encT (512) | W_a (128) | V_a (1) | pad (3)
    #   packB [128, 512] = decT partition-packed (256) | U_a partition-packed (256)
    #   encP  [128, 512] = enc partition-packed: encP[p, 128*i+e] = enc[128*i+p, e]
    packA_d = nc.dram_tensor("packA", [128, TE + HE + 4], BF16, kind="ExternalInput").ap()
    packB_d = nc.dram_tensor("packB", [128, 4 * 128], BF16, kind="ExternalInput").ap()
    encP_d = nc.dram_tensor("encP", [128, S_TILES * HE], F32, kind="ExternalInput").ap()
    c_d = nc.dram_tensor("c", [TD_CORE, HE], F32, kind="ExternalOutput").ap()
    e_d = nc.dram_tensor("e", [TD_CORE, TE], F32, kind="ExternalOutput").ap()

    with tile.TileContext(nc) as tc:
        with ExitStack() as ctx:
            consts = ctx.enter_context(tc.tile_pool(name="consts", bufs=1))
            zpool = ctx.enter_context(tc.tile_pool(name="zpool", bufs=6))
            fpool = ctx.enter_context(tc.tile_pool(name="fpool", bufs=3))
            opool = ctx.enter_context(tc.tile_pool(name="opool", bufs=2))
            ps_e = ctx.enter_context(tc.tile_pool(name="ps_e", bufs=1, space="PSUM"))
            ps_out = ctx.enter_context(
                tc.tile_pool(name="ps_out", bufs=2, space="PSUM")
            )

            # ---- load inputs: 3 packed DMAs ----
            packA = consts.tile([128, TE + HE + 4], BF16, tag="packA")
            nc.sync.dma_start(out=packA[:], in_=packA_d[:])
            packB = consts.tile([128, 4 * 128], BF16, tag="packB")
            nc.scalar.dma_start(out=packB[:], in_=packB_d[:])
            encP = consts.tile([128, S_TILES * HE], F32, tag="encP")
            nc.sync.dma_start(out=encP[:], in_=encP_d[:])

            encT_sb = packA[:, 0:TE]  # [e, s]
            wa_sb = packA[:, TE : TE + HE]
            va_sb = packA[:, TE + HE : TE + HE + 1]
            decT_sb = [packB[:, 0:128], packB[:, 128:256]]  # [d, t] halves
            ua_sb = [packB[:, 256:384], packB[:, 384:512]]  # [d, f] halves
            enc_sb = [encP[:, 128 * i : 128 * (i + 1)] for i in range(S_TILES)]

            ident = consts.tile([128, 128], F32, tag="ident")
            make_identity(nc, ident[:])

            v_bf = va_sb  # already bf16 in packA

            wst_sb = consts.tile([HE, TE], BF16, tag="wst_sb")  # [f, s]
            uht_sb = consts.tile([HE, TD_CORE], F32, tag="uht_sb")  # [f, t]
            with tc.tile_pool(name="ps_pre", bufs=1, space="PSUM") as ps_pre:
                wst_ps = ps_pre.tile([HE, TE], F32, tag="wst")
                nc.tensor.matmul(
                    out=wst_ps[:],
                    lhsT=wa_sb,
                    rhs=encT_sb,
                    start=True,
                    stop=True,
                )
                nc.scalar.copy(out=wst_sb[:], in_=wst_ps[:])

                uht_ps = ps_pre.tile([HE, TD_CORE], F32, tag="uht")
                for i in range(HD // 128):
                    nc.tensor.matmul(
                        out=uht_ps[:],
                        lhsT=ua_sb[i],
                        rhs=decT_sb[i],
                        start=(i == 0),
                        stop=(i == HD // 128 - 1),
                    )
                nc.scalar.copy(out=uht_sb[:], in_=uht_ps[:])

            # ---- per-quarter main loop + epilogue ----
            for q, (tq0, TQ, blocks) in enumerate(QUARTERS):
                # e_ps free layout: index i*TQ + tl holds eT[s in tile i, tq0+tl]
                e_ps = ps_e.tile([128, S_TILES * TQ], F32, tag=f"e_ps{q % 2}")
                tl = 0
                for tb in blocks:
                    z = zpool.tile([128, tb * TE], BF16, tag="z")
                    for j in range(tb):
                        nc.vector.tensor_scalar_add(
                            out=z[:, TE * j : TE * (j + 1)],
                            in0=wst_sb[:],
                            scalar1=uht_sb[:, tq0 + tl + j : tq0 + tl + j + 1],
                        )
                    feat = fpool.tile([128, tb * TE], BF16, tag="feat")
                    nc.scalar.activation(out=feat[:], in_=z[:], func=AF.Tanh)
                    for j in range(tb):
                        for i in range(S_TILES):
                            col = TQ * i + tl + j
                            nc.tensor.matmul(
                                out=e_ps[:, col : col + 1],
                                lhsT=feat[
                                    :, TE * j + 128 * i : TE * j + 128 * (i + 1)
                                ],
                                rhs=v_bf,
                                start=True,
                                stop=True,
                            )
                    tl += tb

                # epilogue for this quarter
                expT = opool.tile([128, S_TILES * TQ], F32, tag="expT")
                nc.scalar.activation(out=expT[:], in_=e_ps[:], func=AF.Exp)

                exp_ts = ps_out.tile([TQ, TE], F32, tag="exp_ts")
                for i in range(S_TILES):
                    nc.tensor.transpose(
                        out=exp_ts[:, 128 * i : 128 * (i + 1)],
                        in_=expT[:, TQ * i : TQ * (i + 1)],
                        identity=ident[:],
                    )

                denom = opool.tile([TQ, 1], F32, tag="denom")
                nc.vector.reduce_sum(
                    out=denom[:], in_=exp_ts[:], axis=mybir.AxisListType.X
                )
                rcol = opool.tile([TQ, 1], F32, tag="rcol")
                nc.vector.reciprocal(out=rcol[:], in_=denom[:])

                e_sm = opool.tile([TQ, TE], F32, tag="e_sm")
                nc.vector.tensor_scalar_mul(
                    out=e_sm[:], in0=exp_ts[:], scalar1=rcol[:]
                )
                nc.sync.dma_start(out=e_d[tq0 : tq0 + TQ, :], in_=e_sm[:])

                c_ps = ps_out.tile([TQ, HE], F32, tag="c_ps")
                for i in range(S_TILES):
                    nc.tensor.matmul(
                        out=c_ps[:],
                        lhsT=expT[:, TQ * i : TQ * (i + 1)],
                        rhs=enc_sb[i],
                        start=(i == 0),
                        stop=(i == S_TILES - 1),
                    )
                c_sb = opool.tile([TQ, HE], F32, tag="c_sb")
                nc.vector.tensor_scalar_mul(
                    out=c_sb[:], in0=c_ps[:], scalar1=rcol[:]
                )
                nc.sync.dma_start(out=c_d[tq0 : tq0 + TQ, :], in_=c_sb[:])

    nc.compile()
    return nc


_NC = None


def _get_nc():
    global _NC
    if _NC is None:
        _NC = _build()
    return _NC


def make_in_maps(encoder_out_seq, decoder_out_seq, W_a, U_a, V_a):
    enc = np.ascontiguousarray(np.asarray(encoder_out_seq, dtype=np.float32))
    dec = np.ascontiguousarray(np.asarray(decoder_out_seq, dtype=np.float32))
    wa = np.ascontiguousarray(np.asarray(W_a, dtype=np.float32))
    ua = np.ascontiguousarray(np.asarray(U_a, dtype=np.float32))
    va = np.ascontiguousarray(np.asarray(V_a, dtype=np.float32))
    in_maps = []
    for core in range(N_CORES):
        b, h = core // 2, core % 2
        dec_slice = dec[b, h * TD_CORE : (h + 1) * TD_CORE, :]
        decT = dec_slice.T  # [256 d, 128 t]
        packA = np.concatenate(
            [enc[b].T, wa, va, np.zeros((HE, 3), np.float32)], axis=1
        ).astype(ml_dtypes.bfloat16)  # [128, 644]
        packB = np.concatenate(
            [
                decT.reshape(2, 128, TD_CORE).transpose(1, 0, 2).reshape(128, 256),
                ua.reshape(2, 128, HE).transpose(1, 0, 2).reshape(128, 256),
            ],
            axis=1,
        ).astype(ml_dtypes.bfloat16)  # [128, 512]
        encP = (
            enc[b].reshape(S_TILES, 128, HE).transpose(1, 0, 2).reshape(128, -1)
        )  # [128, 512]
        in_maps.append(
            {
                "packA": np.ascontiguousarray(packA),
                "packB": np.ascontiguousarray(packB),
                "encP": np.ascontiguousarray(encP),
            }
        )
    return in_maps


def assemble(results):
    c = np.zeros((B, TD, HE), dtype=np.float32)
    e = np.zeros((B, TD, TE), dtype=np.float32)
    for core in range(N_CORES):
        b, h = core // 2, core % 2
        c[b, h * TD_CORE : (h + 1) * TD_CORE, :] = results[core]["c"]
        e[b, h * TD_CORE : (h + 1) * TD_CORE, :] = results[core]["e"]
    return c, e


def kernel(encoder_out_seq, decoder_out_seq, W_a, U_a, V_a):
    nc = _get_nc()
    in_maps = make_in_maps(encoder_out_seq, decoder_out_seq, W_a, U_a, V_a)
    res = run_bass_kernel_spmd(nc, in_maps, list(range(N_CORES)))
    return assemble(res.results)
